# revision 17
# baseline (speedup 1.0000x reference)
"""Causal depthwise Conv1d (K=4) on 8 Trainium2 NeuronCores.

Problem: x (8, 8192, 1024) f32, W (4, 1, 1024) f32, b (1024,) f32
         y[n, t, f] = b[f] + sum_k W[k, 0, f] * x[n, t - 3 + k, f]   (zero pad t<0)

Strategy: data-parallel over batch (8 batches -> 8 cores, no collectives).
Per core, natural layout tiles (partition = time, free = feature):
  - 4 scaled copies tmp_k = x * W[k, :]  (elementwise, DVE + GpSimd)
  - time-shifts + tap-sum as TensorE matmuls with shifted-diagonal lhsT
    matrices S_k, accumulating the 4 taps into PSUM
  - PSUM -> SBUF eviction on ScalarE, DMA out on sync (HWDGE)
Bias is added host-side (it is zero in this problem, but handled anyway).
"""

import os

import numpy as np

B, T, F = 8, 8192, 1024
K = 4
PAD = K - 1
P_OUT = 125          # output rows per main tile (input tile = P_OUT + PAD <= 128)
N_CORES = 8

# compute dtype for x tiles / replicated weights / shift matrices / matmuls.
# float32 = exact; bfloat16 = ~2x DVE throughput + half the inbound DMA.
COMPUTE_DT = os.environ.get("CONV_COMPUTE_DT", "bfloat16")

_BUILD_CACHE = {}
LAST_RESULT = None
LAST_EXEC_NS = None


def _tile_plan(t_total):
    """List of (t0, p_out, in_lo, p_in, is_first)."""
    tiles = [(0, min(P_OUT, t_total), 0, min(P_OUT, t_total), True)]
    t0 = tiles[0][1]
    while t0 < t_total:
        p_out = min(P_OUT, t_total - t0)
        tiles.append((t0, p_out, t0 - PAD, p_out + PAD, False))
        t0 += p_out
    return tiles


def _build(t_total, compute_dt_name):
    import concourse.bacc as bacc
    import concourse.bass as bass
    import concourse.mybir as mybir
    import concourse.tile as tile

    DT = getattr(mybir.dt, compute_dt_name)
    F32 = mybir.dt.float32

    nc = bacc.Bacc("TRN2", target_bir_lowering=False, debug=False)

    x_ext = nc.declare_dram_parameter("x", [t_total, F], DT, isOutput=False)
    wb_ext = nc.declare_dram_parameter("wb", [128, K, F], DT, isOutput=False)
    s_ext = nc.declare_dram_parameter("s", [128, K, 128], DT, isOutput=False)
    s0_ext = nc.declare_dram_parameter("s0", [128, K, 128], DT, isOutput=False)
    out_ext = nc.declare_dram_parameter("out", [t_total, F], F32, isOutput=True)

    tiles = _tile_plan(t_total)
    HALF = F // 2

    with tile.TileContext(nc) as tc:
        with (
            tc.tile_pool(name="const", bufs=1) as cpool,
            tc.tile_pool(name="xin", bufs=6) as xpool,
            tc.tile_pool(name="tmp", bufs=6) as tpool,
            tc.tile_pool(name="yout", bufs=4) as ypool,
            tc.tile_pool(name="ps", bufs=4, space=bass.MemorySpace.PSUM) as pspool,
        ):
            wb = cpool.tile([128, K, F], DT)
            nc.sync.dma_start(wb[:], wb_ext[:])
            smat_reg = cpool.tile([128, K, 128], DT)
            nc.sync.dma_start(smat_reg[:], s_ext[:])
            smat_first = cpool.tile([128, K, 128], DT)
            nc.sync.dma_start(smat_first[:], s0_ext[:])

            # Group tiles so each tap's stationary matrix is loaded once per
            # group and streams 2*G consecutive matmuls: back-to-back same-
            # weight MMs pipeline in the PE array (~213 ns/MM) while a weight
            # reload between MMs forces the full isolated latency (~514 ns).
            # Tile 0 (different lhsT) and the short last tile group alone.
            G = 3
            groups = []
            i = 0
            while i < len(tiles):
                if tiles[i][4] or tiles[i][1] != P_OUT:
                    groups.append([tiles[i]])
                    i += 1
                else:
                    j = i
                    while (
                        j < len(tiles)
                        and len(tiles[i:j + 1]) <= G
                        and not tiles[j][4]
                        and tiles[j][1] == P_OUT
                    ):
                        j += 1
                    groups.append(tiles[i:j])
                    i = j

            for group in groups:
                tmps = {}
                pss = {}
                for gi, (t0, p_out, in_lo, p_in, is_first) in enumerate(group):
                    xt = xpool.tile([128, F], DT, tag="xt")
                    nc.sync.dma_start(xt[0:p_in, :], x_ext[in_lo:in_lo + p_in, :])
                    for k in range(K):
                        tk = tpool.tile([128, F], DT, tag=f"tmp{k}", name=f"tmp{k}")
                        # all taps on DVE: GpSimd tensor_tensor contends with
                        # DVE on the shared SBUF port (measured 3.2x slowdown)
                        nc.vector.tensor_mul(
                            tk[0:p_in, :], xt[0:p_in, :], wb[0:p_in, k, :]
                        )
                        tmps[(gi, k)] = tk
                    pss[gi] = pspool.tile([P_OUT, F], F32, tag="psum", name="psg")

                for k in range(K):
                    for gi, (t0, p_out, in_lo, p_in, is_first) in enumerate(group):
                        smat = smat_first if is_first else smat_reg
                        for h in range(2):
                            nc.tensor.matmul(
                                pss[gi][0:p_out, h * HALF:(h + 1) * HALF],
                                smat[0:p_in, k, 0:p_out],
                                tmps[(gi, k)][0:p_in, h * HALF:(h + 1) * HALF],
                                start=(k == 0),
                                stop=(k == K - 1),
                            )

                for gi, (t0, p_out, in_lo, p_in, is_first) in enumerate(group):
                    yt = ypool.tile([P_OUT, F], F32, tag="yt")
                    nc.scalar.copy(yt[0:p_out, :], pss[gi][0:p_out, :])
                    # SWDGE (gpsimd) out-DMA: its per-partition descriptor
                    # swizzle spreads a 125-partition store across all 16 SDMA
                    # engines; the HWDGE RTL path splits the outer dim evenly
                    # and only reaches 5 engines for 125 rows (125 = 5*25).
                    nc.gpsimd.dma_start(out_ext[t0:t0 + p_out, :], yt[0:p_out, :])

    nc.compile()
    return nc


STRIP = 2048         # time columns per strip in the transposed design
NBANK = 512          # matmul free size = one PSUM bank of fp32


def _build_c(t_total, out_dt_name="bfloat16"):
    """Design C: host passes x transposed (F, T) in bf16. Features sit on
    partitions, so each tap is ONE diagonal matmul per 128-feature block:
      psum[f, t] += diag(W[k, fblock]) @ x_T[fblock, t - 3 + k]
    The time shift is a free-dim AP offset into the strip; PSUM accumulates
    the 4 taps; DVE/ACT only evict PSUM -> SBUF; host transposes the output
    back. No elementwise multiply stage at all.
    """
    import concourse.bacc as bacc
    import concourse.bass as bass
    import concourse.mybir as mybir
    import concourse.tile as tile

    BF16 = mybir.dt.bfloat16
    F32 = mybir.dt.float32
    ODT = getattr(mybir.dt, out_dt_name)

    nc = bacc.Bacc("TRN2", target_bir_lowering=False, debug=False)

    x_ext = nc.declare_dram_parameter("x", [F, t_total], BF16, isOutput=False)
    # diag weights: dw[p, b*K + k, m] = W[k, b*128 + p] iff p == m else 0
    dw_ext = nc.declare_dram_parameter("dw", [128, (F // 128) * K, 128], BF16,
                                       isOutput=False)
    out_ext = nc.declare_dram_parameter("out", [F, t_total], ODT, isOutput=True)

    n_fb = F // 128
    n_strips = (t_total + STRIP - 1) // STRIP
    assert t_total % STRIP == 0
    n_j = STRIP // NBANK

    with tile.TileContext(nc) as tc:
        with (
            tc.tile_pool(name="constc", bufs=1) as cpool,
            tc.tile_pool(name="xs", bufs=4) as xpool,
            tc.tile_pool(name="ys", bufs=3) as ypool,
            tc.tile_pool(name="psc", bufs=8, space=bass.MemorySpace.PSUM) as pspool,
        ):
            dw = cpool.tile([128, n_fb * K, 128], BF16)
            # qAct HWDGE ring keeps the weight load off the qSP ring (x strips
            # go there); f-block 0's matrices come in a small first transfer so
            # the first matmul doesn't wait for the full megabyte.
            nc.scalar.dma_start(dw[:, 0:K, :], dw_ext[:, 0:K, :])
            nc.scalar.dma_start(dw[:, K:, :], dw_ext[:, K:, :])

            for b in range(n_fb):
                frow = b * 128
                for s in range(n_strips):
                    xs = xpool.tile([128, STRIP + PAD], BF16, tag="xs")
                    if s == 0 and b == 0:
                        # first strip arrives in bank-sized chunks so the very
                        # first matmul starts after ~1/4 of the transfer
                        nc.vector.memset(xs[:, 0:PAD], 0.0)
                        for j in range(n_j):
                            nc.sync.dma_start(
                                xs[:, PAD + j * NBANK:PAD + (j + 1) * NBANK],
                                x_ext[frow:frow + 128,
                                      j * NBANK:(j + 1) * NBANK],
                            )
                    elif s == 0:
                        nc.vector.memset(xs[:, 0:PAD], 0.0)
                        nc.sync.dma_start(
                            xs[:, PAD:PAD + STRIP],
                            x_ext[frow:frow + 128, 0:STRIP],
                        )
                    else:
                        nc.sync.dma_start(
                            xs[:, :],
                            x_ext[frow:frow + 128,
                                  s * STRIP - PAD:(s + 1) * STRIP],
                        )
                    pss = {}
                    for j in range(n_j):
                        pss[j] = pspool.tile([128, NBANK], F32, tag="psc",
                                             name="psc")
                    for k in range(K):
                        for j in range(n_j):
                            nc.tensor.matmul(
                                pss[j][:, :],
                                dw[:, b * K + k, :],
                                xs[:, j * NBANK + k:j * NBANK + k + NBANK],
                                start=(k == 0),
                                stop=(k == K - 1),
                            )
                    ys = ypool.tile([128, STRIP], ODT, tag="ys")
                    for j in range(n_j):
                        if j % 2 == 0:
                            nc.scalar.copy(ys[:, j * NBANK:(j + 1) * NBANK],
                                           pss[j][:, :])
                        else:
                            nc.vector.tensor_copy(
                                ys[:, j * NBANK:(j + 1) * NBANK], pss[j][:, :])
                    # 128-partition store fans across all 16 engines on the
                    # HWDGE path, and avoids SWDGE Q7 descriptor-gen which the
                    # DVE eviction-casts (2-port mode) would starve.
                    if b == n_fb - 1 and s == n_strips - 1:
                        # last strip leaves in bank-sized chunks to shorten the
                        # kernel tail behind the final eviction
                        for j in range(n_j):
                            nc.sync.dma_start(
                                out_ext[frow:frow + 128,
                                        s * STRIP + j * NBANK:
                                        s * STRIP + (j + 1) * NBANK],
                                ys[:, j * NBANK:(j + 1) * NBANK],
                            )
                    else:
                        nc.sync.dma_start(
                            out_ext[frow:frow + 128,
                                    s * STRIP:(s + 1) * STRIP],
                            ys[:, :],
                        )

    nc.compile()
    return nc


def _host_constants_c(W):
    import ml_dtypes
    Wk = np.asarray(W, dtype=np.float32).reshape(K, F)
    n_fb = F // 128
    dw = np.zeros((128, n_fb * K, 128), dtype=ml_dtypes.bfloat16)
    for b in range(n_fb):
        for k in range(K):
            dvals = Wk[k, b * 128:(b + 1) * 128].astype(ml_dtypes.bfloat16)
            np.fill_diagonal(dw[:, b * K + k, :], dvals)
    return dw


def _run_c(x_all, W, b, t_total, out_dt_name="bfloat16"):
    import ml_dtypes
    from concourse.bass_utils import run_bass_kernel_spmd

    global LAST_RESULT, LAST_EXEC_NS
    _install_axon_ntff_hook()
    key = ("c", t_total, out_dt_name)
    if key not in _BUILD_CACHE:
        _BUILD_CACHE[key] = _build_c(t_total, out_dt_name)
    nc = _BUILD_CACHE[key]
    dw = _host_constants_c(W)

    in_maps = []
    for i in range(x_all.shape[0]):
        xt = np.ascontiguousarray(
            x_all[i].T.astype(ml_dtypes.bfloat16)
        )  # (F, T) bf16
        in_maps.append({"x": xt, "dw": dw})

    res = run_bass_kernel_spmd(nc, in_maps, core_ids=list(range(len(in_maps))))
    LAST_RESULT = res
    LAST_EXEC_NS = res.exec_time_ns

    outs = []
    for i in range(len(in_maps)):
        o = np.asarray(res.results[i]["out"], dtype=np.float32)  # (F, T)
        outs.append(o.T)  # (T, F)
    out = np.stack(outs, axis=0)
    out = out + np.asarray(b, dtype=np.float32)[None, None, :]
    return np.ascontiguousarray(out.astype(np.float32))


def _get_nc(t_total, compute_dt_name):
    key = (t_total, compute_dt_name)
    if key not in _BUILD_CACHE:
        _BUILD_CACHE[key] = _build(t_total, compute_dt_name)
    return _BUILD_CACHE[key]


def _np_dt(compute_dt_name):
    if compute_dt_name == "bfloat16":
        import ml_dtypes
        return ml_dtypes.bfloat16
    return np.float32


def _host_constants(W, compute_dt_name):
    """Replicated weights (128, K, F) and shift matrices (128, K, 128)."""
    np_dt = _np_dt(compute_dt_name)
    Wk = np.asarray(W, dtype=np.float32).reshape(K, F)
    wb = np.ascontiguousarray(
        np.broadcast_to(Wk[None, :, :], (128, K, F)).astype(np_dt)
    )
    # Regular tiles: input partition p holds time t0 - PAD + p; output row m is
    # time t0 + m; tap k reads x[t0 + m - PAD + k] -> p = m + k.
    s = np.zeros((128, K, 128), dtype=np_dt)
    # First tile: input partition p holds time p; tap k of output m reads
    # x[m - PAD + k] -> p = m + k - PAD, rows with p < 0 are the causal zero pad.
    s0 = np.zeros((128, K, 128), dtype=np_dt)
    for k in range(K):
        for m in range(P_OUT):
            s[m + k, k, m] = 1
            p = m + k - PAD
            if p >= 0:
                s0[p, k, m] = 1
    return wb, s, s0


def _install_axon_ntff_hook():
    """Provide antenv.axon_hooks (absent in this image) so BASS_TRACE=1 can
    capture NTFF profiles through the axon PJRT .so. No-op if present."""
    import contextlib
    import ctypes
    import sys
    import types

    try:
        import antenv.axon_hooks  # noqa: F401
        return
    except ImportError:
        pass

    mod = types.ModuleType("antenv.axon_hooks")
    _state = {"hook": None}
    mod.set_axon_ntff_profile_hook = lambda h: _state.__setitem__("hook", h)
    mod.get_axon_ntff_profile_hook = lambda: _state["hook"]
    try:
        import antenv
        antenv.axon_hooks = mod
    except ImportError:
        pass
    sys.modules["antenv.axon_hooks"] = mod

    try:
        lib = ctypes.CDLL("/opt/axon/libaxon_pjrt.so")
    except OSError:
        return
    if not hasattr(lib, "axon_start_nrt_profile"):
        return
    lib.axon_start_nrt_profile.argtypes = [
        ctypes.POINTER(ctypes.c_int64),
        ctypes.c_size_t,
    ]
    lib.axon_start_nrt_profile.restype = ctypes.c_int64
    lib.axon_stop_nrt_profile.argtypes = [ctypes.c_char_p]
    lib.axon_stop_nrt_profile.restype = ctypes.c_int64

    @contextlib.contextmanager
    def _hook(output_dir, device_ids):
        import jax
        jax.devices()
        if device_ids:
            ids = (ctypes.c_int64 * len(device_ids))(*device_ids)
            rc = lib.axon_start_nrt_profile(ids, len(device_ids))
        else:
            rc = lib.axon_start_nrt_profile(None, 0)
        if rc != 0:
            raise RuntimeError(f"axon_start_nrt_profile rc={rc}")
        try:
            yield
        finally:
            n = lib.axon_stop_nrt_profile(str(output_dir).encode())
            print(f"profile: {n} file(s) written to {output_dir}", file=sys.stderr)

    mod.set_axon_ntff_profile_hook(_hook)


def _run(x_all, W, b, t_total, compute_dt_name):
    from concourse.bass_utils import run_bass_kernel_spmd

    _install_axon_ntff_hook()

    global LAST_RESULT, LAST_EXEC_NS
    np_dt = _np_dt(compute_dt_name)
    nc = _get_nc(t_total, compute_dt_name)
    wb, s, s0 = _host_constants(W, compute_dt_name)

    in_maps = []
    for i in range(N_CORES):
        in_maps.append({
            "x": np.ascontiguousarray(x_all[i].astype(np_dt)),
            "wb": wb,
            "s": s,
            "s0": s0,
        })

    res = run_bass_kernel_spmd(nc, in_maps, core_ids=list(range(N_CORES)))
    LAST_RESULT = res
    LAST_EXEC_NS = res.exec_time_ns

    out = np.stack([res.results[i]["out"] for i in range(N_CORES)], axis=0)
    out = out + np.asarray(b, dtype=np.float32)[None, None, :]
    return np.ascontiguousarray(out.astype(np.float32))


DESIGN = os.environ.get("CONV_DESIGN", "c")


def kernel(x, W, b):
    x = np.asarray(x)
    assert x.shape == (B, T, F), x.shape
    if DESIGN == "c":
        return _run_c(x, W, b, T)
    return _run(x, W, b, T, COMPUTE_DT)


# revision 18
# speedup vs baseline: 1.0291x; 1.0291x over previous
"""Causal depthwise Conv1d (K=4) on 8 Trainium2 NeuronCores.

Problem: x (8, 8192, 1024) f32, W (4, 1, 1024) f32, b (1024,) f32
         y[n, t, f] = b[f] + sum_k W[k, 0, f] * x[n, t - 3 + k, f]   (zero pad t<0)

Strategy: data-parallel over batch (8 batches -> 8 cores, no collectives).
Per core, natural layout tiles (partition = time, free = feature):
  - 4 scaled copies tmp_k = x * W[k, :]  (elementwise, DVE + GpSimd)
  - time-shifts + tap-sum as TensorE matmuls with shifted-diagonal lhsT
    matrices S_k, accumulating the 4 taps into PSUM
  - PSUM -> SBUF eviction on ScalarE, DMA out on sync (HWDGE)
Bias is added host-side (it is zero in this problem, but handled anyway).
"""

import os

import numpy as np

B, T, F = 8, 8192, 1024
K = 4
PAD = K - 1
P_OUT = 125          # output rows per main tile (input tile = P_OUT + PAD <= 128)
N_CORES = 8

# compute dtype for x tiles / replicated weights / shift matrices / matmuls.
# float32 = exact; bfloat16 = ~2x DVE throughput + half the inbound DMA.
COMPUTE_DT = os.environ.get("CONV_COMPUTE_DT", "bfloat16")

_BUILD_CACHE = {}
LAST_RESULT = None
LAST_EXEC_NS = None


def _tile_plan(t_total):
    """List of (t0, p_out, in_lo, p_in, is_first)."""
    tiles = [(0, min(P_OUT, t_total), 0, min(P_OUT, t_total), True)]
    t0 = tiles[0][1]
    while t0 < t_total:
        p_out = min(P_OUT, t_total - t0)
        tiles.append((t0, p_out, t0 - PAD, p_out + PAD, False))
        t0 += p_out
    return tiles


def _build(t_total, compute_dt_name):
    import concourse.bacc as bacc
    import concourse.bass as bass
    import concourse.mybir as mybir
    import concourse.tile as tile

    DT = getattr(mybir.dt, compute_dt_name)
    F32 = mybir.dt.float32

    nc = bacc.Bacc("TRN2", target_bir_lowering=False, debug=False)

    x_ext = nc.declare_dram_parameter("x", [t_total, F], DT, isOutput=False)
    wb_ext = nc.declare_dram_parameter("wb", [128, K, F], DT, isOutput=False)
    s_ext = nc.declare_dram_parameter("s", [128, K, 128], DT, isOutput=False)
    s0_ext = nc.declare_dram_parameter("s0", [128, K, 128], DT, isOutput=False)
    out_ext = nc.declare_dram_parameter("out", [t_total, F], F32, isOutput=True)

    tiles = _tile_plan(t_total)
    HALF = F // 2

    with tile.TileContext(nc) as tc:
        with (
            tc.tile_pool(name="const", bufs=1) as cpool,
            tc.tile_pool(name="xin", bufs=6) as xpool,
            tc.tile_pool(name="tmp", bufs=6) as tpool,
            tc.tile_pool(name="yout", bufs=4) as ypool,
            tc.tile_pool(name="ps", bufs=4, space=bass.MemorySpace.PSUM) as pspool,
        ):
            wb = cpool.tile([128, K, F], DT)
            nc.sync.dma_start(wb[:], wb_ext[:])
            smat_reg = cpool.tile([128, K, 128], DT)
            nc.sync.dma_start(smat_reg[:], s_ext[:])
            smat_first = cpool.tile([128, K, 128], DT)
            nc.sync.dma_start(smat_first[:], s0_ext[:])

            # Group tiles so each tap's stationary matrix is loaded once per
            # group and streams 2*G consecutive matmuls: back-to-back same-
            # weight MMs pipeline in the PE array (~213 ns/MM) while a weight
            # reload between MMs forces the full isolated latency (~514 ns).
            # Tile 0 (different lhsT) and the short last tile group alone.
            G = 3
            groups = []
            i = 0
            while i < len(tiles):
                if tiles[i][4] or tiles[i][1] != P_OUT:
                    groups.append([tiles[i]])
                    i += 1
                else:
                    j = i
                    while (
                        j < len(tiles)
                        and len(tiles[i:j + 1]) <= G
                        and not tiles[j][4]
                        and tiles[j][1] == P_OUT
                    ):
                        j += 1
                    groups.append(tiles[i:j])
                    i = j

            for group in groups:
                tmps = {}
                pss = {}
                for gi, (t0, p_out, in_lo, p_in, is_first) in enumerate(group):
                    xt = xpool.tile([128, F], DT, tag="xt")
                    nc.sync.dma_start(xt[0:p_in, :], x_ext[in_lo:in_lo + p_in, :])
                    for k in range(K):
                        tk = tpool.tile([128, F], DT, tag=f"tmp{k}", name=f"tmp{k}")
                        # all taps on DVE: GpSimd tensor_tensor contends with
                        # DVE on the shared SBUF port (measured 3.2x slowdown)
                        nc.vector.tensor_mul(
                            tk[0:p_in, :], xt[0:p_in, :], wb[0:p_in, k, :]
                        )
                        tmps[(gi, k)] = tk
                    pss[gi] = pspool.tile([P_OUT, F], F32, tag="psum", name="psg")

                for k in range(K):
                    for gi, (t0, p_out, in_lo, p_in, is_first) in enumerate(group):
                        smat = smat_first if is_first else smat_reg
                        for h in range(2):
                            nc.tensor.matmul(
                                pss[gi][0:p_out, h * HALF:(h + 1) * HALF],
                                smat[0:p_in, k, 0:p_out],
                                tmps[(gi, k)][0:p_in, h * HALF:(h + 1) * HALF],
                                start=(k == 0),
                                stop=(k == K - 1),
                            )

                for gi, (t0, p_out, in_lo, p_in, is_first) in enumerate(group):
                    yt = ypool.tile([P_OUT, F], F32, tag="yt")
                    nc.scalar.copy(yt[0:p_out, :], pss[gi][0:p_out, :])
                    # SWDGE (gpsimd) out-DMA: its per-partition descriptor
                    # swizzle spreads a 125-partition store across all 16 SDMA
                    # engines; the HWDGE RTL path splits the outer dim evenly
                    # and only reaches 5 engines for 125 rows (125 = 5*25).
                    nc.gpsimd.dma_start(out_ext[t0:t0 + p_out, :], yt[0:p_out, :])

    nc.compile()
    return nc


STRIP = 2048         # time columns per strip in the transposed design
NBANK = 512          # matmul free size = one PSUM bank of fp32


def _build_c(t_total, out_dt_name="bfloat16"):
    """Design C: host passes x transposed (F, T) in bf16. Features sit on
    partitions, so each tap is ONE diagonal matmul per 128-feature block:
      psum[f, t] += diag(W[k, fblock]) @ x_T[fblock, t - 3 + k]
    The time shift is a free-dim AP offset into the strip; PSUM accumulates
    the 4 taps; DVE/ACT only evict PSUM -> SBUF; host transposes the output
    back. No elementwise multiply stage at all.
    """
    import concourse.bacc as bacc
    import concourse.bass as bass
    import concourse.mybir as mybir
    import concourse.tile as tile

    BF16 = mybir.dt.bfloat16
    F32 = mybir.dt.float32
    ODT = getattr(mybir.dt, out_dt_name)

    nc = bacc.Bacc("TRN2", target_bir_lowering=False, debug=False)

    x_ext = nc.declare_dram_parameter("x", [F, t_total], BF16, isOutput=False)
    # diag weights: dw[p, b*K + k, m] = W[k, b*128 + p] iff p == m else 0
    dw_ext = nc.declare_dram_parameter("dw", [128, (F // 128) * K, 128], BF16,
                                       isOutput=False)
    out_ext = nc.declare_dram_parameter("out", [F, t_total], ODT, isOutput=True)

    n_fb = F // 128
    n_strips = (t_total + STRIP - 1) // STRIP
    assert t_total % STRIP == 0
    n_j = STRIP // NBANK

    with tile.TileContext(nc) as tc:
        with (
            tc.tile_pool(name="constc", bufs=1) as cpool,
            tc.tile_pool(name="xs", bufs=6) as xpool,
            tc.tile_pool(name="ys", bufs=3) as ypool,
            tc.tile_pool(name="psc", bufs=8, space=bass.MemorySpace.PSUM) as pspool,
        ):
            dw = cpool.tile([128, n_fb * K, 128], BF16)
            # qAct HWDGE ring keeps the weight load off the qSP ring (x strips
            # go there); f-block 0's matrices come in a small first transfer so
            # the first matmul doesn't wait for the full megabyte.
            nc.scalar.dma_start(dw[:, 0:K, :], dw_ext[:, 0:K, :])
            nc.scalar.dma_start(dw[:, K:, :], dw_ext[:, K:, :])

            for b in range(n_fb):
                frow = b * 128
                for s in range(n_strips):
                    xs = xpool.tile([128, STRIP + PAD], BF16, tag="xs")
                    if s == 0:
                        nc.vector.memset(xs[:, 0:PAD], 0.0)
                        nc.sync.dma_start(
                            xs[:, PAD:PAD + STRIP],
                            x_ext[frow:frow + 128, 0:STRIP],
                        )
                    else:
                        nc.sync.dma_start(
                            xs[:, :],
                            x_ext[frow:frow + 128,
                                  s * STRIP - PAD:(s + 1) * STRIP],
                        )
                    pss = {}
                    for j in range(n_j):
                        pss[j] = pspool.tile([128, NBANK], F32, tag="psc",
                                             name="psc")
                    for k in range(K):
                        for j in range(n_j):
                            nc.tensor.matmul(
                                pss[j][:, :],
                                dw[:, b * K + k, :],
                                xs[:, j * NBANK + k:j * NBANK + k + NBANK],
                                start=(k == 0),
                                stop=(k == K - 1),
                            )
                    ys = ypool.tile([128, STRIP], ODT, tag="ys")
                    for j in range(n_j):
                        if j % 2 == 0:
                            nc.scalar.copy(ys[:, j * NBANK:(j + 1) * NBANK],
                                           pss[j][:, :])
                        else:
                            nc.vector.tensor_copy(
                                ys[:, j * NBANK:(j + 1) * NBANK], pss[j][:, :])
                    # 128-partition store fans across all 16 engines on the
                    # HWDGE path, and avoids SWDGE Q7 descriptor-gen which the
                    # DVE eviction-casts (2-port mode) would starve.
                    nc.sync.dma_start(
                        out_ext[frow:frow + 128, s * STRIP:(s + 1) * STRIP],
                        ys[:, :],
                    )

    nc.compile()
    return nc


def _host_constants_c(W):
    import ml_dtypes
    Wk = np.asarray(W, dtype=np.float32).reshape(K, F)
    n_fb = F // 128
    dw = np.zeros((128, n_fb * K, 128), dtype=ml_dtypes.bfloat16)
    for b in range(n_fb):
        for k in range(K):
            dvals = Wk[k, b * 128:(b + 1) * 128].astype(ml_dtypes.bfloat16)
            np.fill_diagonal(dw[:, b * K + k, :], dvals)
    return dw


def _run_c(x_all, W, b, t_total, out_dt_name="bfloat16"):
    import ml_dtypes
    from concourse.bass_utils import run_bass_kernel_spmd

    global LAST_RESULT, LAST_EXEC_NS
    _install_axon_ntff_hook()
    key = ("c", t_total, out_dt_name)
    if key not in _BUILD_CACHE:
        _BUILD_CACHE[key] = _build_c(t_total, out_dt_name)
    nc = _BUILD_CACHE[key]
    dw = _host_constants_c(W)

    in_maps = []
    for i in range(x_all.shape[0]):
        xt = np.ascontiguousarray(
            x_all[i].T.astype(ml_dtypes.bfloat16)
        )  # (F, T) bf16
        in_maps.append({"x": xt, "dw": dw})

    res = run_bass_kernel_spmd(nc, in_maps, core_ids=list(range(len(in_maps))))
    LAST_RESULT = res
    LAST_EXEC_NS = res.exec_time_ns

    outs = []
    for i in range(len(in_maps)):
        o = np.asarray(res.results[i]["out"], dtype=np.float32)  # (F, T)
        outs.append(o.T)  # (T, F)
    out = np.stack(outs, axis=0)
    out = out + np.asarray(b, dtype=np.float32)[None, None, :]
    return np.ascontiguousarray(out.astype(np.float32))


def _get_nc(t_total, compute_dt_name):
    key = (t_total, compute_dt_name)
    if key not in _BUILD_CACHE:
        _BUILD_CACHE[key] = _build(t_total, compute_dt_name)
    return _BUILD_CACHE[key]


def _np_dt(compute_dt_name):
    if compute_dt_name == "bfloat16":
        import ml_dtypes
        return ml_dtypes.bfloat16
    return np.float32


def _host_constants(W, compute_dt_name):
    """Replicated weights (128, K, F) and shift matrices (128, K, 128)."""
    np_dt = _np_dt(compute_dt_name)
    Wk = np.asarray(W, dtype=np.float32).reshape(K, F)
    wb = np.ascontiguousarray(
        np.broadcast_to(Wk[None, :, :], (128, K, F)).astype(np_dt)
    )
    # Regular tiles: input partition p holds time t0 - PAD + p; output row m is
    # time t0 + m; tap k reads x[t0 + m - PAD + k] -> p = m + k.
    s = np.zeros((128, K, 128), dtype=np_dt)
    # First tile: input partition p holds time p; tap k of output m reads
    # x[m - PAD + k] -> p = m + k - PAD, rows with p < 0 are the causal zero pad.
    s0 = np.zeros((128, K, 128), dtype=np_dt)
    for k in range(K):
        for m in range(P_OUT):
            s[m + k, k, m] = 1
            p = m + k - PAD
            if p >= 0:
                s0[p, k, m] = 1
    return wb, s, s0


def _install_axon_ntff_hook():
    """Provide antenv.axon_hooks (absent in this image) so BASS_TRACE=1 can
    capture NTFF profiles through the axon PJRT .so. No-op if present."""
    import contextlib
    import ctypes
    import sys
    import types

    try:
        import antenv.axon_hooks  # noqa: F401
        return
    except ImportError:
        pass

    mod = types.ModuleType("antenv.axon_hooks")
    _state = {"hook": None}
    mod.set_axon_ntff_profile_hook = lambda h: _state.__setitem__("hook", h)
    mod.get_axon_ntff_profile_hook = lambda: _state["hook"]
    try:
        import antenv
        antenv.axon_hooks = mod
    except ImportError:
        pass
    sys.modules["antenv.axon_hooks"] = mod

    try:
        lib = ctypes.CDLL("/opt/axon/libaxon_pjrt.so")
    except OSError:
        return
    if not hasattr(lib, "axon_start_nrt_profile"):
        return
    lib.axon_start_nrt_profile.argtypes = [
        ctypes.POINTER(ctypes.c_int64),
        ctypes.c_size_t,
    ]
    lib.axon_start_nrt_profile.restype = ctypes.c_int64
    lib.axon_stop_nrt_profile.argtypes = [ctypes.c_char_p]
    lib.axon_stop_nrt_profile.restype = ctypes.c_int64

    @contextlib.contextmanager
    def _hook(output_dir, device_ids):
        import jax
        jax.devices()
        if device_ids:
            ids = (ctypes.c_int64 * len(device_ids))(*device_ids)
            rc = lib.axon_start_nrt_profile(ids, len(device_ids))
        else:
            rc = lib.axon_start_nrt_profile(None, 0)
        if rc != 0:
            raise RuntimeError(f"axon_start_nrt_profile rc={rc}")
        try:
            yield
        finally:
            n = lib.axon_stop_nrt_profile(str(output_dir).encode())
            print(f"profile: {n} file(s) written to {output_dir}", file=sys.stderr)

    mod.set_axon_ntff_profile_hook(_hook)


def _run(x_all, W, b, t_total, compute_dt_name):
    from concourse.bass_utils import run_bass_kernel_spmd

    _install_axon_ntff_hook()

    global LAST_RESULT, LAST_EXEC_NS
    np_dt = _np_dt(compute_dt_name)
    nc = _get_nc(t_total, compute_dt_name)
    wb, s, s0 = _host_constants(W, compute_dt_name)

    in_maps = []
    for i in range(N_CORES):
        in_maps.append({
            "x": np.ascontiguousarray(x_all[i].astype(np_dt)),
            "wb": wb,
            "s": s,
            "s0": s0,
        })

    res = run_bass_kernel_spmd(nc, in_maps, core_ids=list(range(N_CORES)))
    LAST_RESULT = res
    LAST_EXEC_NS = res.exec_time_ns

    out = np.stack([res.results[i]["out"] for i in range(N_CORES)], axis=0)
    out = out + np.asarray(b, dtype=np.float32)[None, None, :]
    return np.ascontiguousarray(out.astype(np.float32))


DESIGN = os.environ.get("CONV_DESIGN", "c")


def kernel(x, W, b):
    x = np.asarray(x)
    assert x.shape == (B, T, F), x.shape
    if DESIGN == "c":
        return _run_c(x, W, b, T)
    return _run(x, W, b, T, COMPUTE_DT)
